# revision 13
# baseline (speedup 1.0000x reference)
"""Bahdanau attention Trainium2 kernel (8-core data-parallel).

Per core (256 batches, S=200, A=128, K=V=256), 32 rounds x 8 batches:
  - keys (host-cast fp16) loaded transposed via xbar DMA -> [k_part, (b s)]
  - kproj = WkT.T @ keysT on PE (fp16, 2-batch pairs, free=400)
  - energy = tanh(kproj + Wq@q + biases) fused on ScalarE (fp16 out)
  - scores = v_w.T @ energy on PE; softmax with batch-in-partition layout
  - context = attn.T @ values on PE (fp16 stationary attn columns)
Outputs fp32: context [B,256], attn_weights [B,200].
"""
import numpy as np
import concourse.bass as bass
import concourse.bacc as bacc
import concourse.mybir as mybir
from concourse.tile import TileContext
from concourse.masks import make_identity
from concourse.bass_utils import run_bass_kernel_spmd

f16 = mybir.dt.float16
f32 = mybir.dt.float32
AF = mybir.ActivationFunctionType

NCORES = 8
B, S, KD, VD, AD = 2048, 200, 256, 256, 128
BC = B // NCORES            # 256 batches per core
RB = 32                     # batches per round
NR = BC // RB               # 32 rounds
ROWS = RB * S               # 1600 key/value rows per round
VPAD = 56                   # values padding rows (per-batch 256-row loads)

_CACHE = {}


def _build(ablate=()):
    nc = bacc.Bacc("TRN2")
    keys16 = nc.dram_tensor("keys16", [BC * S, KD], f16, kind="ExternalInput")
    vals16 = nc.dram_tensor("vals16", [BC * S + VPAD, VD], f16, kind="ExternalInput")
    qT16 = nc.dram_tensor("qT16", [KD, BC], f16, kind="ExternalInput")
    wqT16 = nc.dram_tensor("wqT16", [KD, AD], f16, kind="ExternalInput")
    wkT16 = nc.dram_tensor("wkT16", [KD, AD], f16, kind="ExternalInput")
    bias2 = nc.dram_tensor("bias2", [AD, 1], f32, kind="ExternalInput")
    vw16 = nc.dram_tensor("vw16", [AD, 1], f16, kind="ExternalInput")
    ctx_out = nc.dram_tensor("ctx_out", [BC, VD], f32, kind="ExternalOutput")
    attn_out = nc.dram_tensor("attn_out", [BC, S], f32, kind="ExternalOutput")

    with TileContext(nc) as tc:
        with (
            tc.tile_pool(name="consts", bufs=1) as consts,
            tc.tile_pool(name="kpool", bufs=2) as kpool,
            tc.tile_pool(name="vpool", bufs=2) as vpool,
            tc.tile_pool(name="enpool", bufs=4) as enpool,
            tc.tile_pool(name="stpool", bufs=1) as stpool,
            tc.tile_pool(name="softp", bufs=3) as softp,
            tc.tile_pool(name="psk", bufs=2, space="PSUM") as psk_pool,
            tc.tile_pool(name="pss", bufs=2, space="PSUM") as pss_pool,
            tc.tile_pool(name="psc", bufs=2, space="PSUM") as psc_pool,
            tc.tile_pool(name="psm", bufs=2, space="PSUM") as psm_pool,
        ):
            # ---- constants / setup ----
            wq_sb = consts.tile([128, 2, AD], f16, tag="wq")
            nc.sync.dma_start(out=wq_sb[:], in_=wqT16.rearrange("(e p) a -> p e a", p=128))
            wk_sb = consts.tile([128, 2, AD], f16, tag="wk")
            nc.sync.dma_start(out=wk_sb[:], in_=wkT16.rearrange("(e p) a -> p e a", p=128))
            vw_sb = consts.tile([128, 1], f16, tag="vw")
            nc.sync.dma_start(out=vw_sb[:], in_=vw16[:])
            bias_sb = consts.tile([128, 1], f32, tag="bias")
            nc.sync.dma_start(out=bias_sb[:], in_=bias2[:])
            qT_sb = consts.tile([128, 2, BC], f16, tag="qT")
            nc.sync.dma_start(out=qT_sb[:], in_=qT16.rearrange("(e p) b -> p e b", p=128))
            ident = consts.tile([128, 128], f32, tag="ident")
            make_identity(nc, ident[:])

            # qproj for all 256 batches at once -> qpb[a, b] = Wq@q + (Wq_b + Wk_b)
            ps_q = psm_pool.tile([128, BC], f32, tag="misc")
            nc.tensor.matmul(ps_q[:], wq_sb[:, 0, :], qT_sb[:, 0, :], start=True, stop=False)
            nc.tensor.matmul(ps_q[:], wq_sb[:, 1, :], qT_sb[:, 1, :], start=False, stop=True)
            qpb = consts.tile([128, BC], f32, tag="qpb")
            nc.vector.tensor_scalar_add(qpb[:], ps_q[:], bias_sb[:])

            # ---- main loop ----
            for t in range(NR):
                r0 = t * ROWS
                kT = kpool.tile([128, 2, ROWS], f16, tag="kT")
                nc.sync.dma_start(out=kT[:], in_=keys16[r0:r0 + ROWS, :], transpose=True)

                if "vals" not in ablate:
                    vtile = vpool.tile([128, RB, 2, VD], f16, tag="val")
                    # overlapping 4D AP: row (b, two, p) = t*1600 + b*200 + two*128 + p
                    for two in range(2):
                        src_ap = bass.AP(
                            vals16, (t * ROWS + two * 128) * VD,
                            [[VD, 128], [S * VD, RB], [1, VD]],
                        )
                        nc.sync.dma_start(out=vtile[:, :, two, :], in_=src_ap)

                stage = stpool.tile([1, RB * S], f32, tag="sstage")
                for p in range(0 if "kproj" in ablate else RB // 2):
                    c0 = p * 2 * S
                    psk = psk_pool.tile([128, 2 * S], f32, tag="kproj")
                    nc.tensor.matmul(psk[:], wk_sb[:, 0, :], kT[:, 0, c0:c0 + 2 * S],
                                     start=True, stop=False)
                    nc.tensor.matmul(psk[:], wk_sb[:, 1, :], kT[:, 1, c0:c0 + 2 * S],
                                     start=False, stop=True)
                    en = enpool.tile([128, 2 * S], f16, tag="energy")
                    b0 = t * RB + 2 * p
                    nc.scalar.activation(en[:, 0:S], psk[:, 0:S], AF.Tanh,
                                         bias=qpb[:, b0:b0 + 1], scale=1.0)
                    nc.scalar.activation(en[:, S:2 * S], psk[:, S:2 * S], AF.Tanh,
                                         bias=qpb[:, b0 + 1:b0 + 2], scale=1.0)
                    pss = pss_pool.tile([1, 2 * S], f32, tag="scores")
                    nc.tensor.matmul(pss[:], vw_sb[:], en[:], start=True, stop=True)
                    if p % 2 == 0:
                        nc.scalar.activation(stage[0:1, c0:c0 + 2 * S], pss[:], AF.Copy)
                    else:
                        nc.vector.tensor_copy(stage[0:1, c0:c0 + 2 * S], pss[:])

                # softmax over 8 batches (batch-in-partition layout)
                if "soft" in ablate:
                    continue
                ssb = softp.tile([RB, S], f32, tag="ssb")
                nc.sync.dma_start(out=ssb[:], in_=stage[:])
                nmax = softp.tile([RB, 1], f32, tag="nmax")
                nc.vector.reduce_max(nmax[:], ssb[:], axis=mybir.AxisListType.X, negate=True)
                eub = softp.tile([RB, S], f32, tag="eub")
                sume = softp.tile([RB, 1], f32, tag="sume")
                nc.scalar.activation(eub[:], ssb[:], AF.Exp,
                                     bias=nmax[:], scale=1.0, accum_out=sume[:])
                rsum = softp.tile([RB, 1], f32, tag="rsum")
                nc.vector.reciprocal(rsum[:], sume[:])
                attn = softp.tile([RB, S], f32, tag="attn")
                nc.vector.tensor_scalar_mul(attn[:], eub[:], rsum[:])
                nc.sync.dma_start(out=attn_out[t * RB:(t + 1) * RB, :], in_=attn[:])

                # transpose attn -> [s, b] fp16 columns
                pst = psm_pool.tile([128, 2 * RB], f32, tag="misc")
                nc.tensor.transpose(pst[:, 0:RB], attn[:, 0:128], ident[0:RB, 0:RB])
                nc.tensor.transpose(pst[0:72, RB:2 * RB], attn[:, 128:S], ident[0:RB, 0:RB])
                aT = softp.tile([128, 2 * RB], f16, tag="aT")
                nc.vector.tensor_copy(aT[:, 0:RB], pst[:, 0:RB])
                nc.vector.tensor_copy(aT[0:72, RB:2 * RB], pst[0:72, RB:2 * RB])

                # context: per batch, attn-column stationary x values
                if "ctx" in ablate or "vals" in ablate:
                    continue
                cstage = stpool.tile([1, RB * VD], f32, tag="cstage")
                for p in range(RB // 2):
                    cp = p
                    psc = psc_pool.tile([1, 2 * VD], f32, tag="ctx")
                    for h in range(2):
                        b = 2 * p + h
                        nc.tensor.matmul(psc[0:1, h * VD:(h + 1) * VD],
                                         aT[:, b:b + 1], vtile[:, b, 0, :],
                                         start=True, stop=False)
                        nc.tensor.matmul(psc[0:1, h * VD:(h + 1) * VD],
                                         aT[0:72, RB + b:RB + b + 1], vtile[0:72, b, 1, :],
                                         start=False, stop=True)
                    if p % 2 == 0:
                        nc.vector.tensor_copy(cstage[0:1, cp * 2 * VD:(cp + 1) * 2 * VD], psc[:])
                    else:
                        nc.scalar.activation(cstage[0:1, cp * 2 * VD:(cp + 1) * 2 * VD], psc[:], AF.Copy)
                nc.sync.dma_start(out=ctx_out[t * RB:(t + 1) * RB, :], in_=cstage[:])

    nc.finalize()
    return nc


def _prep_inputs(query, keys, values, Wq_w, Wq_b, Wk_w, Wk_b, v_w):
    query = np.asarray(query, np.float32)
    keys = np.asarray(keys, np.float32)
    values = np.asarray(values, np.float32)
    wqT16 = np.ascontiguousarray(np.asarray(Wq_w, np.float32).T).astype(np.float16)
    wkT16 = np.ascontiguousarray(np.asarray(Wk_w, np.float32).T).astype(np.float16)
    bias2 = (np.asarray(Wq_b, np.float32) + np.asarray(Wk_b, np.float32)).reshape(AD, 1)
    vw16 = np.asarray(v_w, np.float16).reshape(AD, 1)
    in_maps = []
    for c in range(NCORES):
        sl = slice(c * BC, (c + 1) * BC)
        k16 = keys[sl].reshape(BC * S, KD).astype(np.float16)
        v16 = values[sl].reshape(BC * S, VD).astype(np.float16)
        v16 = np.concatenate([v16, np.zeros((VPAD, VD), np.float16)], axis=0)
        qT = np.ascontiguousarray(query[sl].T).astype(np.float16)
        in_maps.append({
            "keys16": np.ascontiguousarray(k16),
            "vals16": np.ascontiguousarray(v16),
            "qT16": qT,
            "wqT16": wqT16,
            "wkT16": wkT16,
            "bias2": np.ascontiguousarray(bias2, np.float32),
            "vw16": vw16,
        })
    return in_maps


def kernel(query, keys, values, Wq_w, Wq_b, Wk_w, Wk_b, v_w, _trace=False):
    if "nc" not in _CACHE:
        _CACHE["nc"] = _build()
    nc = _CACHE["nc"]
    in_maps = _prep_inputs(query, keys, values, Wq_w, Wq_b, Wk_w, Wk_b, v_w)
    res = run_bass_kernel_spmd(nc, in_maps, list(range(NCORES)), trace=_trace)
    kernel.last_result = res
    context = np.concatenate([res.results[c]["ctx_out"] for c in range(NCORES)], axis=0)
    attn = np.concatenate([res.results[c]["attn_out"] for c in range(NCORES)], axis=0)
    return context, attn


# revision 15
# speedup vs baseline: 1.0050x; 1.0050x over previous
"""Bahdanau attention Trainium2 kernel (8-core data-parallel).

Per core (256 batches, S=200, A=128, K=V=256), 32 rounds x 8 batches:
  - keys (host-cast fp16) loaded transposed via xbar DMA -> [k_part, (b s)]
  - kproj = WkT.T @ keysT on PE (fp16, 2-batch pairs, free=400)
  - energy = tanh(kproj + Wq@q + biases) fused on ScalarE (fp16 out)
  - scores = v_w.T @ energy on PE; softmax with batch-in-partition layout
  - context = attn.T @ values on PE (fp16 stationary attn columns)
Outputs fp32: context [B,256], attn_weights [B,200].
"""
import numpy as np
import concourse.bass as bass
import concourse.bacc as bacc
import concourse.mybir as mybir
from concourse.tile import TileContext
from concourse.masks import make_identity
from concourse.bass_utils import run_bass_kernel_spmd

f16 = mybir.dt.float16
f32 = mybir.dt.float32
AF = mybir.ActivationFunctionType

NCORES = 8
B, S, KD, VD, AD = 2048, 200, 256, 256, 128
BC = B // NCORES            # 256 batches per core
RB = 32                     # batches per round
NR = BC // RB               # 32 rounds
ROWS = RB * S               # 1600 key/value rows per round
VPAD = 56                   # values padding rows (per-batch 256-row loads)

_CACHE = {}


def _build(ablate=()):
    nc = bacc.Bacc("TRN2")
    keys16 = nc.dram_tensor("keys16", [BC * S, KD], f16, kind="ExternalInput")
    vals16 = nc.dram_tensor("vals16", [BC * S + VPAD, VD], f16, kind="ExternalInput")
    qT16 = nc.dram_tensor("qT16", [KD, BC], f16, kind="ExternalInput")
    wqT16 = nc.dram_tensor("wqT16", [KD, AD], f16, kind="ExternalInput")
    wkT16 = nc.dram_tensor("wkT16", [KD, AD], f16, kind="ExternalInput")
    bias2 = nc.dram_tensor("bias2", [AD, 1], f32, kind="ExternalInput")
    vw16 = nc.dram_tensor("vw16", [AD, 1], f16, kind="ExternalInput")
    ctx_out = nc.dram_tensor("ctx_out", [BC, VD], f32, kind="ExternalOutput")
    attn_out = nc.dram_tensor("attn_out", [BC, S], f32, kind="ExternalOutput")

    with TileContext(nc) as tc:
        with (
            tc.tile_pool(name="consts", bufs=1) as consts,
            tc.tile_pool(name="kpool", bufs=2) as kpool,
            tc.tile_pool(name="vpool", bufs=2) as vpool,
            tc.tile_pool(name="enpool", bufs=4) as enpool,
            tc.tile_pool(name="stpool", bufs=1) as stpool,
            tc.tile_pool(name="softp", bufs=3) as softp,
            tc.tile_pool(name="psk", bufs=2, space="PSUM") as psk_pool,
            tc.tile_pool(name="pss", bufs=2, space="PSUM") as pss_pool,
            tc.tile_pool(name="psc", bufs=2, space="PSUM") as psc_pool,
            tc.tile_pool(name="psm", bufs=2, space="PSUM") as psm_pool,
        ):
            # ---- constants / setup ----
            wq_sb = consts.tile([128, 2, AD], f16, tag="wq")
            nc.sync.dma_start(out=wq_sb[:], in_=wqT16.rearrange("(e p) a -> p e a", p=128))
            wk_sb = consts.tile([128, 2, AD], f16, tag="wk")
            nc.sync.dma_start(out=wk_sb[:], in_=wkT16.rearrange("(e p) a -> p e a", p=128))
            vw_sb = consts.tile([128, 1], f16, tag="vw")
            nc.sync.dma_start(out=vw_sb[:], in_=vw16[:])
            bias_sb = consts.tile([128, 1], f32, tag="bias")
            nc.sync.dma_start(out=bias_sb[:], in_=bias2[:])
            qT_sb = consts.tile([128, 2, BC], f16, tag="qT")
            nc.sync.dma_start(out=qT_sb[:], in_=qT16.rearrange("(e p) b -> p e b", p=128))
            ident = consts.tile([128, 128], f32, tag="ident")
            make_identity(nc, ident[:])

            # qproj for all 256 batches at once -> qpb[a, b] = Wq@q + (Wq_b + Wk_b)
            ps_q = psm_pool.tile([128, BC], f32, tag="misc")
            nc.tensor.matmul(ps_q[:], wq_sb[:, 0, :], qT_sb[:, 0, :], start=True, stop=False)
            nc.tensor.matmul(ps_q[:], wq_sb[:, 1, :], qT_sb[:, 1, :], start=False, stop=True)
            qpb = consts.tile([128, BC], f32, tag="qpb")
            nc.vector.tensor_scalar_add(qpb[:], ps_q[:], bias_sb[:])

            # ---- main loop (variable round sizes: small head/tail) ----
            schedule = [16, 32, 32, 32, 32, 32, 32, 32, 16]
            assert sum(schedule) == BC
            b_off = 0
            for rb in schedule:
                rows = rb * S
                r0 = b_off * S
                kT = kpool.tile([128, 2, rows], f16, tag="kT")
                nc.sync.dma_start(out=kT[:], in_=keys16[r0:r0 + rows, :], transpose=True)

                if "vals" not in ablate:
                    vtile = vpool.tile([128, rb, 2, VD], f16, tag="val")
                    for two in range(2):
                        src_ap = bass.AP(
                            vals16, (r0 + two * 128) * VD,
                            [[VD, 128], [S * VD, rb], [1, VD]],
                        )
                        nc.sync.dma_start(out=vtile[:, :, two, :], in_=src_ap)

                stage = stpool.tile([1, rb * S], f32, tag="sstage")
                for p in range(0 if "kproj" in ablate else rb // 2):
                    c0 = p * 2 * S
                    psk = psk_pool.tile([128, 2 * S], f32, tag="kproj")
                    nc.tensor.matmul(psk[:], wk_sb[:, 0, :], kT[:, 0, c0:c0 + 2 * S],
                                     start=True, stop=False)
                    nc.tensor.matmul(psk[:], wk_sb[:, 1, :], kT[:, 1, c0:c0 + 2 * S],
                                     start=False, stop=True)
                    en = enpool.tile([128, 2 * S], f16, tag="energy")
                    b0 = b_off + 2 * p
                    nc.scalar.activation(en[:, 0:S], psk[:, 0:S], AF.Tanh,
                                         bias=qpb[:, b0:b0 + 1], scale=1.0)
                    nc.scalar.activation(en[:, S:2 * S], psk[:, S:2 * S], AF.Tanh,
                                         bias=qpb[:, b0 + 1:b0 + 2], scale=1.0)
                    pss = pss_pool.tile([1, 2 * S], f32, tag="scores")
                    nc.tensor.matmul(pss[:], vw_sb[:], en[:], start=True, stop=True)
                    if p % 2 == 0:
                        nc.scalar.activation(stage[0:1, c0:c0 + 2 * S], pss[:], AF.Copy)
                    else:
                        nc.vector.tensor_copy(stage[0:1, c0:c0 + 2 * S], pss[:])

                # softmax over 8 batches (batch-in-partition layout)
                if "soft" in ablate:
                    continue
                ssb = softp.tile([rb, S], f32, tag="ssb")
                nc.sync.dma_start(out=ssb[:], in_=stage[:])
                nmax = softp.tile([rb, 1], f32, tag="nmax")
                nc.vector.reduce_max(nmax[:], ssb[:], axis=mybir.AxisListType.X, negate=True)
                eub = softp.tile([rb, S], f32, tag="eub")
                sume = softp.tile([rb, 1], f32, tag="sume")
                nc.scalar.activation(eub[:], ssb[:], AF.Exp,
                                     bias=nmax[:], scale=1.0, accum_out=sume[:])
                rsum = softp.tile([rb, 1], f32, tag="rsum")
                nc.vector.reciprocal(rsum[:], sume[:])
                attn = softp.tile([rb, S], f32, tag="attn")
                nc.vector.tensor_scalar_mul(attn[:], eub[:], rsum[:])
                nc.sync.dma_start(out=attn_out[b_off:b_off + rb, :], in_=attn[:])

                # transpose attn -> [s, b] fp16 columns
                pst = psm_pool.tile([128, 2 * rb], f32, tag="misc")
                nc.tensor.transpose(pst[:, 0:rb], attn[:, 0:128], ident[0:rb, 0:rb])
                nc.tensor.transpose(pst[0:72, rb:2 * rb], attn[:, 128:S], ident[0:rb, 0:rb])
                aT = softp.tile([128, 2 * rb], f16, tag="aT")
                nc.vector.tensor_copy(aT[:, 0:rb], pst[:, 0:rb])
                nc.vector.tensor_copy(aT[0:72, rb:2 * rb], pst[0:72, rb:2 * rb])

                # context: per batch, attn-column stationary x values
                if "ctx" in ablate or "vals" in ablate:
                    continue
                cstage = stpool.tile([1, rb * VD], f32, tag="cstage")
                for p in range(rb // 2):
                    cp = p
                    psc = psc_pool.tile([1, 2 * VD], f32, tag="ctx")
                    for h in range(2):
                        b = 2 * p + h
                        nc.tensor.matmul(psc[0:1, h * VD:(h + 1) * VD],
                                         aT[:, b:b + 1], vtile[:, b, 0, :],
                                         start=True, stop=False)
                        nc.tensor.matmul(psc[0:1, h * VD:(h + 1) * VD],
                                         aT[0:72, rb + b:rb + b + 1], vtile[0:72, b, 1, :],
                                         start=False, stop=True)
                    if p % 2 == 0:
                        nc.vector.tensor_copy(cstage[0:1, cp * 2 * VD:(cp + 1) * 2 * VD], psc[:])
                    else:
                        nc.scalar.activation(cstage[0:1, cp * 2 * VD:(cp + 1) * 2 * VD], psc[:], AF.Copy)
                nc.sync.dma_start(out=ctx_out[b_off:b_off + rb, :], in_=cstage[:])
                b_off += rb

    nc.finalize()
    return nc


def _prep_inputs(query, keys, values, Wq_w, Wq_b, Wk_w, Wk_b, v_w):
    query = np.asarray(query, np.float32)
    keys = np.asarray(keys, np.float32)
    values = np.asarray(values, np.float32)
    wqT16 = np.ascontiguousarray(np.asarray(Wq_w, np.float32).T).astype(np.float16)
    wkT16 = np.ascontiguousarray(np.asarray(Wk_w, np.float32).T).astype(np.float16)
    bias2 = (np.asarray(Wq_b, np.float32) + np.asarray(Wk_b, np.float32)).reshape(AD, 1)
    vw16 = np.asarray(v_w, np.float16).reshape(AD, 1)
    in_maps = []
    for c in range(NCORES):
        sl = slice(c * BC, (c + 1) * BC)
        k16 = keys[sl].reshape(BC * S, KD).astype(np.float16)
        v16 = values[sl].reshape(BC * S, VD).astype(np.float16)
        v16 = np.concatenate([v16, np.zeros((VPAD, VD), np.float16)], axis=0)
        qT = np.ascontiguousarray(query[sl].T).astype(np.float16)
        in_maps.append({
            "keys16": np.ascontiguousarray(k16),
            "vals16": np.ascontiguousarray(v16),
            "qT16": qT,
            "wqT16": wqT16,
            "wkT16": wkT16,
            "bias2": np.ascontiguousarray(bias2, np.float32),
            "vw16": vw16,
        })
    return in_maps


def kernel(query, keys, values, Wq_w, Wq_b, Wk_w, Wk_b, v_w, _trace=False):
    if "nc" not in _CACHE:
        _CACHE["nc"] = _build()
    nc = _CACHE["nc"]
    in_maps = _prep_inputs(query, keys, values, Wq_w, Wq_b, Wk_w, Wk_b, v_w)
    res = run_bass_kernel_spmd(nc, in_maps, list(range(NCORES)), trace=_trace)
    kernel.last_result = res
    context = np.concatenate([res.results[c]["ctx_out"] for c in range(NCORES)], axis=0)
    attn = np.concatenate([res.results[c]["attn_out"] for c in range(NCORES)], axis=0)
    return context, attn


# revision 16
# speedup vs baseline: 1.0191x; 1.0140x over previous
"""Bahdanau attention Trainium2 kernel (8-core data-parallel).

Per core (256 batches, S=200, A=128, K=V=256), 32 rounds x 8 batches:
  - keys (host-cast fp16) loaded transposed via xbar DMA -> [k_part, (b s)]
  - kproj = WkT.T @ keysT on PE (fp16, 2-batch pairs, free=400)
  - energy = tanh(kproj + Wq@q + biases) fused on ScalarE (fp16 out)
  - scores = v_w.T @ energy on PE; softmax with batch-in-partition layout
  - context = attn.T @ values on PE (fp16 stationary attn columns)
Outputs fp32: context [B,256], attn_weights [B,200].
"""
import numpy as np
import concourse.bass as bass
import concourse.bacc as bacc
import concourse.mybir as mybir
from concourse.tile import TileContext
from concourse.masks import make_identity
from concourse.bass_utils import run_bass_kernel_spmd

f16 = mybir.dt.float16
f32 = mybir.dt.float32
AF = mybir.ActivationFunctionType

NCORES = 8
B, S, KD, VD, AD = 2048, 200, 256, 256, 128
BC = B // NCORES            # 256 batches per core
RB = 32                     # batches per round
NR = BC // RB               # 32 rounds
ROWS = RB * S               # 1600 key/value rows per round
VPAD = 56                   # values padding rows (per-batch 256-row loads)

_CACHE = {}


def _build(ablate=()):
    nc = bacc.Bacc("TRN2")
    keys16 = nc.dram_tensor("keys16", [BC * S, KD], f16, kind="ExternalInput")
    vals16 = nc.dram_tensor("vals16", [BC * S + VPAD, VD], f16, kind="ExternalInput")
    qT16 = nc.dram_tensor("qT16", [KD, BC], f16, kind="ExternalInput")
    wqT16 = nc.dram_tensor("wqT16", [KD, AD], f16, kind="ExternalInput")
    wkT16 = nc.dram_tensor("wkT16", [KD, AD], f16, kind="ExternalInput")
    bias2 = nc.dram_tensor("bias2", [AD, 1], f32, kind="ExternalInput")
    vw16 = nc.dram_tensor("vw16", [AD, 1], f16, kind="ExternalInput")
    ctx_out = nc.dram_tensor("ctx_out", [BC, VD], f32, kind="ExternalOutput")
    attn_out = nc.dram_tensor("attn_out", [BC, S], f32, kind="ExternalOutput")

    with TileContext(nc) as tc:
        with (
            tc.tile_pool(name="consts", bufs=1) as consts,
            tc.tile_pool(name="kpool", bufs=2) as kpool,
            tc.tile_pool(name="vpool", bufs=2) as vpool,
            tc.tile_pool(name="enpool", bufs=4) as enpool,
            tc.tile_pool(name="stpool", bufs=1) as stpool,
            tc.tile_pool(name="softp", bufs=3) as softp,
            tc.tile_pool(name="psk", bufs=2, space="PSUM") as psk_pool,
            tc.tile_pool(name="pss", bufs=2, space="PSUM") as pss_pool,
            tc.tile_pool(name="psc", bufs=3, space="PSUM") as psc_pool,
            tc.tile_pool(name="psm", bufs=1, space="PSUM") as psm_pool,
        ):
            # ---- constants / setup ----
            wq_sb = consts.tile([128, 2, AD], f16, tag="wq")
            nc.sync.dma_start(out=wq_sb[:], in_=wqT16.rearrange("(e p) a -> p e a", p=128))
            wk_sb = consts.tile([128, 2, AD], f16, tag="wk")
            nc.sync.dma_start(out=wk_sb[:], in_=wkT16.rearrange("(e p) a -> p e a", p=128))
            vw_sb = consts.tile([128, 1], f16, tag="vw")
            nc.sync.dma_start(out=vw_sb[:], in_=vw16[:])
            bias_sb = consts.tile([128, 1], f32, tag="bias")
            nc.sync.dma_start(out=bias_sb[:], in_=bias2[:])
            qT_sb = consts.tile([128, 2, BC], f16, tag="qT")
            nc.sync.dma_start(out=qT_sb[:], in_=qT16.rearrange("(e p) b -> p e b", p=128))
            ident = consts.tile([128, 128], f32, tag="ident")
            make_identity(nc, ident[:])

            # qproj for all 256 batches at once -> qpb[a, b] = Wq@q + (Wq_b + Wk_b)
            ps_q = psm_pool.tile([128, BC], f32, tag="misc")
            nc.tensor.matmul(ps_q[:], wq_sb[:, 0, :], qT_sb[:, 0, :], start=True, stop=False)
            nc.tensor.matmul(ps_q[:], wq_sb[:, 1, :], qT_sb[:, 1, :], start=False, stop=True)
            qpb = consts.tile([128, BC], f32, tag="qpb")
            nc.vector.tensor_scalar_add(qpb[:], ps_q[:], bias_sb[:])

            # ---- main loop (variable round sizes: small head/tail) ----
            schedule = [16, 32, 32, 32, 32, 32, 32, 32, 16]
            assert sum(schedule) == BC
            b_off = 0
            for rb in schedule:
                rows = rb * S
                r0 = b_off * S
                kT = kpool.tile([128, 2, rows], f16, tag="kT")
                nc.sync.dma_start(out=kT[:], in_=keys16[r0:r0 + rows, :], transpose=True)

                if "vals" not in ablate:
                    vtile = vpool.tile([128, rb, 2, VD], f16, tag="val")
                    for two in range(2):
                        src_ap = bass.AP(
                            vals16, (r0 + two * 128) * VD,
                            [[VD, 128], [S * VD, rb], [1, VD]],
                        )
                        nc.sync.dma_start(out=vtile[:, :, two, :], in_=src_ap)

                stage = stpool.tile([1, rb * S], f32, tag="sstage")
                for p in range(0 if "kproj" in ablate else rb // 2):
                    c0 = p * 2 * S
                    psk = psk_pool.tile([128, 2 * S], f32, tag="kproj")
                    nc.tensor.matmul(psk[:], wk_sb[:, 0, :], kT[:, 0, c0:c0 + 2 * S],
                                     start=True, stop=False)
                    nc.tensor.matmul(psk[:], wk_sb[:, 1, :], kT[:, 1, c0:c0 + 2 * S],
                                     start=False, stop=True)
                    en = enpool.tile([128, 2 * S], f16, tag="energy")
                    b0 = b_off + 2 * p
                    nc.scalar.activation(en[:, 0:S], psk[:, 0:S], AF.Tanh,
                                         bias=qpb[:, b0:b0 + 1], scale=1.0)
                    nc.scalar.activation(en[:, S:2 * S], psk[:, S:2 * S], AF.Tanh,
                                         bias=qpb[:, b0 + 1:b0 + 2], scale=1.0)
                    pss = pss_pool.tile([1, 2 * S], f32, tag="scores")
                    nc.tensor.matmul(pss[:], vw_sb[:], en[:], start=True, stop=True)
                    if p % 2 == 0:
                        nc.scalar.activation(stage[0:1, c0:c0 + 2 * S], pss[:], AF.Copy)
                    else:
                        nc.vector.tensor_copy(stage[0:1, c0:c0 + 2 * S], pss[:])

                # softmax over 8 batches (batch-in-partition layout)
                if "soft" in ablate:
                    continue
                ssb = softp.tile([rb, S], f32, tag="ssb")
                nc.sync.dma_start(out=ssb[:], in_=stage[:])
                nmax = softp.tile([rb, 1], f32, tag="nmax")
                nc.vector.reduce_max(nmax[:], ssb[:], axis=mybir.AxisListType.X, negate=True)
                eub = softp.tile([rb, S], f32, tag="eub")
                sume = softp.tile([rb, 1], f32, tag="sume")
                nc.scalar.activation(eub[:], ssb[:], AF.Exp,
                                     bias=nmax[:], scale=1.0, accum_out=sume[:])
                rsum = softp.tile([rb, 1], f32, tag="rsum")
                nc.vector.reciprocal(rsum[:], sume[:])
                attn = softp.tile([rb, S], f32, tag="attn")
                nc.vector.tensor_scalar_mul(attn[:], eub[:], rsum[:])
                nc.sync.dma_start(out=attn_out[b_off:b_off + rb, :], in_=attn[:])

                # transpose attn -> [s, b] fp16 columns
                pst = psm_pool.tile([128, 2 * rb], f32, tag="misc")
                nc.tensor.transpose(pst[:, 0:rb], attn[:, 0:128], ident[0:rb, 0:rb])
                nc.tensor.transpose(pst[0:72, rb:2 * rb], attn[:, 128:S], ident[0:rb, 0:rb])
                aT = softp.tile([128, 2 * rb], f16, tag="aT")
                nc.vector.tensor_copy(aT[:, 0:rb], pst[:, 0:rb])
                nc.vector.tensor_copy(aT[0:72, rb:2 * rb], pst[0:72, rb:2 * rb])

                # context: per batch, attn-column stationary x values
                if "ctx" in ablate or "vals" in ablate:
                    continue
                cstage = stpool.tile([1, rb * VD], f32, tag="cstage")
                for p in range(rb // 2):
                    cp = p
                    psc = psc_pool.tile([1, 2 * VD], f32, tag="ctx")
                    for h in range(2):
                        b = 2 * p + h
                        nc.tensor.matmul(psc[0:1, h * VD:(h + 1) * VD],
                                         aT[:, b:b + 1], vtile[:, b, 0, :],
                                         start=True, stop=False)
                        nc.tensor.matmul(psc[0:1, h * VD:(h + 1) * VD],
                                         aT[0:72, rb + b:rb + b + 1], vtile[0:72, b, 1, :],
                                         start=False, stop=True)
                    if p % 2 == 0:
                        nc.vector.tensor_copy(cstage[0:1, cp * 2 * VD:(cp + 1) * 2 * VD], psc[:])
                    else:
                        nc.scalar.activation(cstage[0:1, cp * 2 * VD:(cp + 1) * 2 * VD], psc[:], AF.Copy)
                nc.sync.dma_start(out=ctx_out[b_off:b_off + rb, :], in_=cstage[:])
                b_off += rb

    nc.finalize()
    return nc


def _prep_inputs(query, keys, values, Wq_w, Wq_b, Wk_w, Wk_b, v_w):
    query = np.asarray(query, np.float32)
    keys = np.asarray(keys, np.float32)
    values = np.asarray(values, np.float32)
    wqT16 = np.ascontiguousarray(np.asarray(Wq_w, np.float32).T).astype(np.float16)
    wkT16 = np.ascontiguousarray(np.asarray(Wk_w, np.float32).T).astype(np.float16)
    bias2 = (np.asarray(Wq_b, np.float32) + np.asarray(Wk_b, np.float32)).reshape(AD, 1)
    vw16 = np.asarray(v_w, np.float16).reshape(AD, 1)
    in_maps = []
    for c in range(NCORES):
        sl = slice(c * BC, (c + 1) * BC)
        k16 = keys[sl].reshape(BC * S, KD).astype(np.float16)
        v16 = values[sl].reshape(BC * S, VD).astype(np.float16)
        v16 = np.concatenate([v16, np.zeros((VPAD, VD), np.float16)], axis=0)
        qT = np.ascontiguousarray(query[sl].T).astype(np.float16)
        in_maps.append({
            "keys16": np.ascontiguousarray(k16),
            "vals16": np.ascontiguousarray(v16),
            "qT16": qT,
            "wqT16": wqT16,
            "wkT16": wkT16,
            "bias2": np.ascontiguousarray(bias2, np.float32),
            "vw16": vw16,
        })
    return in_maps


def kernel(query, keys, values, Wq_w, Wq_b, Wk_w, Wk_b, v_w, _trace=False):
    if "nc" not in _CACHE:
        _CACHE["nc"] = _build()
    nc = _CACHE["nc"]
    in_maps = _prep_inputs(query, keys, values, Wq_w, Wq_b, Wk_w, Wk_b, v_w)
    res = run_bass_kernel_spmd(nc, in_maps, list(range(NCORES)), trace=_trace)
    kernel.last_result = res
    context = np.concatenate([res.results[c]["ctx_out"] for c in range(NCORES)], axis=0)
    attn = np.concatenate([res.results[c]["attn_out"] for c in range(NCORES)], axis=0)
    return context, attn
